# revision 1
# baseline (speedup 1.0000x reference)
"""Trainium2 Bass kernel for nn_ExpandOperator (banded scatter of a linear projection).

Reference semantics:
    pred = x @ W.T + b                      # (B, S, 2048)
    pred = pred.reshape(B, S, 64, 32)
    out[b, t, (t+s) % S, d] = pred[b, t, s, d]   # rest of out is zeros
    out shape: (B, S, S, 32) fp32  == 1 GiB

Sharding: 8 cores = (batch b in {0,1}) x (512-row seq chunk cc in {0..3}).
Each core computes pred for its 512 rows and writes its 128 MiB output slice.

Core-invariant SPMD trick: every core runs the identical program pretending its
rows are t = 0..511, so the scatter band sits on a fixed diagonal with no
wraparound.  The host rotates each core's block along the S axis by 512*cc
when unsharding (pure block memcpy).

Diagonal layout: the per-core output is declared as (512, 65568) where row t is
[2048-float band | 63520 floats of zeros].  Row-major linearization of this
buffer equals the true (512, 2048, 32) slice (band t lives at flat offset
65536*t + 32*t = 65568*t) plus a 64 KiB tail pad that the host drops.  Band and
gap writes are uniform strided DMAs covering every true output byte exactly
once - no overlapping writes, no ordering hazards.

This walrus build only leaves room for ONE sync-wait per compute instruction,
so everything a matmul/copy depends on must arrive through a single semaphore:
W.T, x.T, the bias AND a ones-row are packed into ONE input tensor "wx" loaded
by a single DMA.  The bias is folded into the matmul contraction itself: wx is
padded from 768 to 896 rows (7 K-tiles of 128), with row 768 = [b | 1.0s] and
rows 769..895 = 0, so `x_aug @ Waug.T` = x @ W.T + b with no extra ops.
"""

import numpy as np

import bass_rust
import concourse.bass as bass
import concourse.mybir as mybir
import concourse.tile as tile
from concourse.bass_utils import run_bass_kernel_spmd

F32 = mybir.dt.float32


def _split_multi_waits(nc):
    """Walrus in this toolchain only leaves ONE sync-wait slot per
    instruction.  Tile's tail drain waits on every semaphore lane it used
    (14 here), which fails codegen.  Hoist all-but-one wait of any multi-wait
    instruction into single-wait NOPs on the same engine queue immediately
    before it - semantically identical (same-queue waits execute in order).
    """
    eng_by_type = {
        mybir.EngineType.SP: nc.sync,
        mybir.EngineType.PE: nc.tensor,
        mybir.EngineType.Activation: nc.scalar,
        mybir.EngineType.Pool: nc.gpsimd,
        mybir.EngineType.DVE: nc.vector,
    }
    tail_bb = nc.cur_bb.bb
    for f in nc.m.functions:
        for bb in f.blocks:
            il = bb.instructions
            i = 0
            while i < len(il):
                ins = il[i]
                si = getattr(ins, "sync_info", None)
                if si is not None and len(si.on_wait) > 1:
                    waits = list(si.on_wait)
                    for w in waits[:-1]:
                        nop = eng_by_type[ins.engine].nop(nofuse=True).ins
                        tail_bb.instructions.remove(nop)
                        nop.sync_info = bass_rust.SyncInfo(
                            on_wait=[w], on_update=[])
                        il.insert(i, nop)
                        i += 1
                    ins.sync_info = bass_rust.SyncInfo(
                        on_wait=[waits[-1]], on_update=list(si.on_update))
                i += 1

# Problem shapes (hardcoded per contract).
B = 2
S = 2048
D_IN = 768
MAX_SPAN = 64
SPAN_DIM = 32
N_OUT = MAX_SPAN * SPAN_DIM  # 2048
N_CORES = 8
CHUNKS = 4                   # seq chunks per batch (B * CHUNKS == N_CORES)
ROWS = S // CHUNKS           # 512 rows per core


def build_nc(rows=ROWS, s=S, d_in=D_IN, n_out=N_OUT, span_dim=SPAN_DIM,
             gap_split=8, repeats=1, opt=False):
    if opt:
        return build_nc_opt(rows, s, d_in, n_out, span_dim, repeats)
    return build_nc_v1(rows, s, d_in, n_out, span_dim, gap_split, repeats)


def build_nc_opt(rows=ROWS, s=S, d_in=D_IN, n_out=N_OUT, span_dim=SPAN_DIM,
                 repeats=1):
    """Optimized variant: half the store-DMA count (63.5KB-contiguous gap
    chunks for blocks 1+, split memset so block 0 starts early), reads only
    the 769 meaningful wx rows (bias via K=1 matmul from a tiny second load),
    and one merged 4MB band DMA."""
    row_f = s * span_dim
    period = row_f + span_dim
    gap = period - n_out
    assert gap % 8 == 0
    gw8 = gap // 8                  # narrow chunk (block 0, early start)
    gw4 = gap // 4                  # wide chunk (blocks 1+)
    d_pad = -(-(d_in + 1) // 128) * 128
    kt = d_in // 128                # full contraction tiles (bias separate)
    mblk = rows // 128
    nw = min(512, n_out)
    nchunk = n_out // nw
    wcols = n_out + rows

    nc = bass.Bass()
    wx = nc.dram_tensor("wx", [d_pad, wcols], F32, kind="ExternalInput")
    out = nc.dram_tensor("out", [rows, period], F32, kind="ExternalOutput")

    wx_r = wx[0:d_in, :].rearrange("(k p) m -> p k m", p=128)
    out_r = out.rearrange("(mb p) c -> p mb c", p=128)  # (128, mblk, period)

    with tile.TileContext(nc) as tc:
        with (
            tc.tile_pool(name="const", bufs=1) as cpool,
            tc.tile_pool(name="pred", bufs=2) as ppool,
            tc.tile_pool(name="psum", bufs=4, space="PSUM") as pspool,
        ):
            zt = cpool.tile([128, gw4], F32)
            nc.vector.memset(zt[:, :gw8], 0.0)   # DVE sem 1
            nc.vector.memset(zt[:, gw8:], 0.0)   # DVE sem 2

            for _rep in range(repeats):
                # Block 0 gaps: 8 narrow chunks, only need the first memset.
                for g in range(8):
                    cs = n_out + g * gw8
                    nc.sync.dma_start(out[0:128, cs:cs + gw8], zt[:, :gw8])
                # Blocks 1+: 4 wide chunks each (63.5 KB contiguous bursts).
                for mb in range(1, mblk):
                    rs = mb * 128
                    for g in range(4):
                        cs = n_out + g * gw4
                        nc.sync.dma_start(out[rs:rs + 128, cs:cs + gw4],
                                          zt[:])

                # Loads: 768 rows of [W.T | x.T] + the single bias/ones row.
                wx_sb = cpool.tile([128, kt, wcols], F32, tag="wx_sb")
                nc.scalar.dma_start(wx_sb[:], wx_r[:])
                bias_sb = cpool.tile([1, wcols], F32, tag="bias_sb")
                nc.scalar.dma_start(bias_sb[:], wx[d_in:d_in + 1, :])

                pred_all = ppool.tile([128, mblk, n_out], F32)
                for mb in range(mblk):
                    rs = mb * 128
                    for n in range(nchunk):
                        ns = n * nw
                        ps = pspool.tile([128, nw], F32)
                        for k in range(kt):
                            nc.tensor.matmul(
                                ps[:],
                                wx_sb[:, k, n_out + rs:n_out + rs + 128],
                                wx_sb[:, k, ns:ns + nw],
                                start=(k == 0),
                                stop=False,
                            )
                        # K=1 bias matmul: ones-row outer bias-row.
                        nc.tensor.matmul(
                            ps[:],
                            bias_sb[:, n_out + rs:n_out + rs + 128],
                            bias_sb[:, ns:ns + nw],
                            start=False,
                            stop=True,
                        )
                        nc.vector.tensor_copy(pred_all[:, mb, ns:ns + nw],
                                              ps[:])
                # One merged band DMA for all blocks.
                nc.gpsimd.dma_start(out_r[:, :, 0:n_out], pred_all[:])

    _split_multi_waits(nc)
    return nc


def build_nc_v1(rows=ROWS, s=S, d_in=D_IN, n_out=N_OUT, span_dim=SPAN_DIM,
                gap_split=8, repeats=1):
    """Build the single-core Bass program (shared by all 8 cores via SPMD).

    Inputs (per core):
      wx : (d_pad, n_out + rows)  [Waug.T | x_aug.T] packed -> one DMA load,
           d_pad = round_up(d_in + 1, 128); row d_in = [b | 1.0s], rest 0.
    Output:
      out: (rows, period) diagonal-layout buffer, period = s*span_dim + span_dim
    """
    row_f = s * span_dim            # true floats per output row
    period = row_f + span_dim       # diagonal period (band marches span_dim/row)
    gap = period - n_out            # zero floats after each band
    assert gap % gap_split == 0
    gw = gap // gap_split           # floats per gap-chunk DMA
    d_pad = -(-(d_in + 1) // 128) * 128
    kt = d_pad // 128               # contraction tiles (incl. bias tile)
    mblk = rows // 128              # 128-row blocks
    nw = min(512, n_out)            # psum chunk width (one fp32 bank)
    nchunk = n_out // nw
    wcols = n_out + rows            # packed free width

    nc = bass.Bass()
    wx = nc.dram_tensor("wx", [d_pad, wcols], F32, kind="ExternalInput")
    out = nc.dram_tensor("out", [rows, period], F32, kind="ExternalOutput")

    wx_r = wx.rearrange("(k p) m -> p k m", p=128)   # (128, kt, wcols)

    with tile.TileContext(nc) as tc:
        with (
            tc.tile_pool(name="const", bufs=1) as cpool,
            tc.tile_pool(name="pred", bufs=mblk) as ppool,
            tc.tile_pool(name="psum", bufs=4, space="PSUM") as pspool,
        ):
            # Zero source tile for the gap writes.
            zt = cpool.tile([128, gw], F32)
            nc.vector.memset(zt[:], 0.0)

            # repeats>1 duplicates the whole body for timing measurements
            # (the dispatch path has a ~650us/call floor that hides the
            # kernel; differencing repeat counts cancels it).
            for _rep in range(repeats):
                # Gap writes: everything after each band, uniform strided
                # DMAs.  These only depend on the memset, so they start
                # immediately.
                for mb in range(mblk):
                    rs = mb * 128
                    for g in range(gap_split):
                        cs = n_out + g * gw
                        nc.sync.dma_start(out[rs:rs + 128, cs:cs + gw], zt[:])

                # Weights + activations + bias row in one DMA (one
                # semaphore).  Issued on the scalar HWDGE ring so it never
                # queues behind the gap stores on the sync ring.
                wx_sb = cpool.tile([128, kt, wcols], F32, tag="wx_sb")
                nc.scalar.dma_start(wx_sb[:], wx_r[:])

                # pred = x @ W.T + b, one 128-row block at a time.
                for mb in range(mblk):
                    rs = mb * 128
                    pt = ppool.tile([128, n_out], F32)
                    for n in range(nchunk):
                        ns = n * nw
                        ps = pspool.tile([128, nw], F32)
                        for k in range(kt):
                            nc.tensor.matmul(
                                ps[:],
                                wx_sb[:, k, n_out + rs:n_out + rs + 128],
                                wx_sb[:, k, ns:ns + nw],
                                start=(k == 0),
                                stop=(k == kt - 1),
                            )
                        # PSUM -> SBUF move (bias folded into matmul).
                        nc.vector.tensor_copy(pt[:, ns:ns + nw], ps[:])
                    # Band write: row t of this block goes to out[t, 0:n_out],
                    # which in flat space is the diagonal 65568*t + [0, 2048).
                    # Issued via SWDGE (gpsimd) whose lanes are otherwise
                    # idle: every instruction here may carry at most ONE sync
                    # wait, and on the sync ring this DMA would need a
                    # lane-FIFO wait on top of its DVE data wait.
                    nc.gpsimd.dma_start(out[rs:rs + 128, 0:n_out], pt[:])

    _split_multi_waits(nc)
    return nc


_CACHE = {}


def _get_nc():
    if "nc" not in _CACHE:
        _CACHE["nc"] = build_nc()
    return _CACHE["nc"]


def make_in_maps(x, W, b):
    """Host-side sharding: per-core packed input dicts."""
    d_pad = -(-(D_IN + 1) // 128) * 128  # 896
    x = x.astype(np.float32, copy=False)
    W = W.astype(np.float32, copy=False)
    b = b.astype(np.float32, copy=False)
    in_maps = []
    for c in range(N_CORES):
        bi, cc = divmod(c, CHUNKS)
        xs = x[bi, cc * ROWS:(cc + 1) * ROWS, :]
        wx_np = np.zeros((d_pad, N_OUT + ROWS), np.float32)
        wx_np[:D_IN, :N_OUT] = W.T
        wx_np[:D_IN, N_OUT:] = xs.T
        wx_np[D_IN, :N_OUT] = b
        wx_np[D_IN, N_OUT:] = 1.0
        in_maps.append({"wx": wx_np})
    return in_maps


def unshard(results):
    """Host-side unsharding: drop tail pad, rotate along S by 512*cc, place."""
    row_f = S * SPAN_DIM
    out = np.empty((B, S, S, SPAN_DIM), np.float32)
    for c in range(N_CORES):
        bi, cc = divmod(c, CHUNKS)
        buf = np.asarray(results[c]["out"])
        local = buf.reshape(-1)[:ROWS * row_f].reshape(ROWS, S, SPAN_DIM)
        sh = cc * ROWS
        blk = out[bi, sh:sh + ROWS]
        if sh:
            blk[:, sh:, :] = local[:, :S - sh, :]
            blk[:, :sh, :] = local[:, S - sh:, :]
        else:
            blk[:, :, :] = local
    return out


def kernel(x, W, b):
    x = np.asarray(x)
    W = np.asarray(W)
    b = np.asarray(b)
    nc = _get_nc()
    res = run_bass_kernel_spmd(nc, make_in_maps(x, W, b),
                               list(range(N_CORES)))
    return unshard(res.results)



# revision 5
# speedup vs baseline: 12.2897x; 12.2897x over previous
"""Trainium2 Bass kernel for nn_ExpandOperator (banded scatter of a linear projection).

Reference semantics:
    pred = x @ W.T + b                      # (B, S, 2048)
    pred = pred.reshape(B, S, 64, 32)
    out[b, t, (t+s) % S, d] = pred[b, t, s, d]   # rest of out is zeros
    out shape: (B, S, S, 32) fp32  == 1 GiB

Sharding: 8 cores = (batch b in {0,1}) x (512-row seq chunk cc in {0..3}).

Key structure: the 1 GiB output is 96.9% structural zeros — only the
(B, S, 2048)-float band carries data, and every band value is just
pred[b, t, :].  So the device computes ONLY the dense projection
pred = x @ W.T for its 512 rows (bf16 in/out; tolerance is 2e-2, bf16
error here is ~1e-3) and returns it as a compact (512, 2048) tile.
The host unshards by scattering the band into an np.zeros output —
row t's band occupies flat columns [32*t, 32*t+2048) mod 65536 of
out[b, t], which for the 1985 non-wrapping rows is a single strided
(diagonal) view assignment; the 63 wrapping rows are split copies.
The bias add (exact fp32) also folds into the host scatter:
out band row = pred_row + b.

Device per core: load [W.T | x.T] packed bf16 (6 k-tiles of 128 rows,
one DMA each so matmuls start after the first ~1.8us), 96 bf16 matmuls
(128x128x512, k-outer over 8 concurrent PSUM banks so accumulation
overlaps the remaining loads), DVE PSUM->SBUF copies (fp32->bf16), and
4 per-row-block band stores.  ~6 MB of HBM traffic and ~20.5us of PE
time per core, vs 134 MB of DMA in the write-the-zeros formulation.

The walrus build only leaves room for ONE sync-wait per compute
instruction; _split_multi_waits() hoists extra waits into same-queue
NOPs (same-queue waits execute in order, so this is semantics-neutral).
"""

import numpy as np

import bass_rust
import concourse.bass as bass
import concourse.mybir as mybir
import concourse.tile as tile
from concourse.bass_utils import run_bass_kernel_spmd

F32 = mybir.dt.float32
BF16 = mybir.dt.bfloat16
NP_BF16 = mybir.dt.np(mybir.dt.bfloat16)


def _split_multi_waits(nc):
    """Walrus in this toolchain only leaves ONE sync-wait slot per
    instruction.  Tile's tail drain waits on every semaphore lane it used,
    which fails codegen.  Hoist all-but-one wait of any multi-wait
    instruction into single-wait NOPs on the same engine queue immediately
    before it - semantically identical (same-queue waits execute in order).
    """
    eng_by_type = {
        mybir.EngineType.SP: nc.sync,
        mybir.EngineType.PE: nc.tensor,
        mybir.EngineType.Activation: nc.scalar,
        mybir.EngineType.Pool: nc.gpsimd,
        mybir.EngineType.DVE: nc.vector,
    }
    tail_bb = nc.cur_bb.bb
    for f in nc.m.functions:
        for bb in f.blocks:
            il = bb.instructions
            i = 0
            while i < len(il):
                ins = il[i]
                si = getattr(ins, "sync_info", None)
                if si is not None and len(si.on_wait) > 1:
                    waits = list(si.on_wait)
                    for w in waits[:-1]:
                        nop = eng_by_type[ins.engine].nop(nofuse=True).ins
                        tail_bb.instructions.remove(nop)
                        nop.sync_info = bass_rust.SyncInfo(
                            on_wait=[w], on_update=[])
                        il.insert(i, nop)
                        i += 1
                    ins.sync_info = bass_rust.SyncInfo(
                        on_wait=[waits[-1]], on_update=list(si.on_update))
                i += 1


# Problem shapes (hardcoded per contract).
B = 2
S = 2048
D_IN = 768
MAX_SPAN = 64
SPAN_DIM = 32
N_OUT = MAX_SPAN * SPAN_DIM  # 2048
N_CORES = 8
CHUNKS = 4                   # seq chunks per batch (B * CHUNKS == N_CORES)
ROWS = S // CHUNKS           # 512 rows per core


def build_nc(rows=ROWS, d_in=D_IN, n_out=N_OUT, repeats=1, nw=512):
    """Single-core Bass program (shared by all 8 cores via SPMD).

    Inputs (per core):
      wx : (d_in, n_out + rows) bf16, packed [W.T | x_chunk.T].
    Output:
      out: (rows, n_out) bf16 = pred = x_chunk @ W.T (no bias; host adds it).
    """
    kt = d_in // 128             # 6 contraction tiles
    mblk = rows // 128           # 4 row blocks
    nchunk = n_out // nw
    wcols = n_out + rows         # 2560
    half_mb = mblk // 2          # row blocks per PSUM generation

    nc = bass.Bass()
    wx = nc.dram_tensor("wx", [d_in, wcols], BF16, kind="ExternalInput")
    out = nc.dram_tensor("out", [rows, n_out], BF16, kind="ExternalOutput")

    wx_r = wx.rearrange("(k p) m -> p k m", p=128)    # (128, kt, wcols)
    out_r = out.rearrange("(mb p) c -> p mb c", p=128)  # (128, mblk, n_out)

    with tile.TileContext(nc) as tc:
        with (
            tc.tile_pool(name="wxp", bufs=2) as wxpool,
            tc.tile_pool(name="pred", bufs=2) as ppool,
            tc.tile_pool(name="psum", bufs=8, space="PSUM") as pspool,
        ):
            for _rep in range(repeats):
                # Per-k-tile loads so the first matmul sweep can start
                # after ~1/6 of the load, overlapping the rest.
                wx_sb = wxpool.tile([128, kt, wcols], BF16)
                for k in range(kt):
                    nc.scalar.dma_start(wx_sb[:, k, :], wx_r[:, k, :])

                pred = ppool.tile([128, mblk, n_out], BF16)
                # Two generations of 8 concurrent PSUM banks; k-outer so
                # accumulation for all 8 chunks proceeds as k-tiles land.
                for half in range(2):
                    pss = [pspool.tile([128, nw], F32, name="ps")
                           for _ in range(half_mb * nchunk)]
                    for k in range(kt):
                        for mi in range(half_mb):
                            mb = half * half_mb + mi
                            cs = n_out + mb * 128
                            for n in range(nchunk):
                                nc.tensor.matmul(
                                    pss[mi * nchunk + n][:],
                                    wx_sb[:, k, cs:cs + 128],
                                    wx_sb[:, k, n * nw:(n + 1) * nw],
                                    start=(k == 0),
                                    stop=(k == kt - 1),
                                )
                    for mi in range(half_mb):
                        mb = half * half_mb + mi
                        for n in range(nchunk):
                            nc.vector.tensor_copy(
                                pred[:, mb, n * nw:(n + 1) * nw],
                                pss[mi * nchunk + n][:])
                        # Band store for this 128-row block (4KB/partition).
                        nc.sync.dma_start(out_r[:, mb, :], pred[:, mb, :])

    _split_multi_waits(nc)
    return nc


_CACHE = {}


def _get_nc():
    if "nc" not in _CACHE:
        _CACHE["nc"] = build_nc()
    return _CACHE["nc"]


def make_in_maps(x, W, b):
    """Host-side sharding: per-core packed [W.T | x_chunk.T] bf16."""
    x = np.asarray(x)
    W = np.asarray(W)
    WT = np.ascontiguousarray(W.T).astype(NP_BF16)    # (768, 2048)
    in_maps = []
    for c in range(N_CORES):
        bi, cc = divmod(c, CHUNKS)
        xs = x[bi, cc * ROWS:(cc + 1) * ROWS, :]
        wxc = np.empty((D_IN, N_OUT + ROWS), NP_BF16)
        wxc[:, :N_OUT] = WT
        wxc[:, N_OUT:] = np.ascontiguousarray(xs.T).astype(NP_BF16)
        in_maps.append({"wx": wxc})
    return in_maps


def unshard(results, b):
    """Scatter each core's dense band into the zero-filled full output.

    Row t's band occupies flat columns [32*t, 32*t+2048) mod 65536 of
    out[bi, t]; rows 0..1984 never wrap, so they're one strided
    (diagonal-view) assignment per batch.  Bias is added here in fp32.
    """
    b = np.asarray(b, dtype=np.float32)
    row_f = S * SPAN_DIM                       # 65536 floats per row
    nowrap = (row_f - N_OUT) // SPAN_DIM + 1   # 1985 non-wrapping rows
    out = np.zeros((B, S, S, SPAN_DIM), np.float32)
    for bi in range(B):
        band = np.concatenate(
            [np.asarray(results[bi * CHUNKS + cc]["out"]).astype(np.float32)
             for cc in range(CHUNKS)], axis=0)          # (2048, 2048)
        band += b[None, :]
        flat = out[bi].reshape(S * row_f)
        dv = np.lib.stride_tricks.as_strided(
            flat, shape=(nowrap, N_OUT),
            strides=((row_f + SPAN_DIM) * 4, 4))
        dv[:] = band[:nowrap]
        for t in range(nowrap, S):
            c0 = SPAN_DIM * t
            n1 = row_f - c0
            row = flat[t * row_f:(t + 1) * row_f]
            row[c0:] = band[t, :n1]
            row[:N_OUT - n1] = band[t, n1:]
    return out


def _run(nc, in_maps):
    return run_bass_kernel_spmd(nc, in_maps, list(range(N_CORES))).results


def kernel(x, W, b):
    x = np.asarray(x)
    W = np.asarray(W)
    b = np.asarray(b)
    nc = _get_nc()
    res = _run(nc, make_in_maps(x, W, b))
    return unshard(res, b)


# revision 6
# speedup vs baseline: 12.8610x; 1.0465x over previous
"""Trainium2 Bass kernel for nn_ExpandOperator (banded scatter of a linear projection).

Reference semantics:
    pred = x @ W.T + b                      # (B, S, 2048)
    pred = pred.reshape(B, S, 64, 32)
    out[b, t, (t+s) % S, d] = pred[b, t, s, d]   # rest of out is zeros
    out shape: (B, S, S, 32) fp32  == 1 GiB

Sharding: 8 cores = (batch b in {0,1}) x (512-row seq chunk cc in {0..3}).

Key structure: the 1 GiB output is 96.9% structural zeros — only the
(B, S, 2048)-float band carries data, and every band value is just
pred[b, t, :].  So the device computes ONLY the dense projection
pred = x @ W.T for its 512 rows (bf16 in/out; tolerance is 2e-2, bf16
error here is ~1e-3) and returns it as a compact (512, 2048) tile.
The host unshards by scattering the band into an np.zeros output —
row t's band occupies flat columns [32*t, 32*t+2048) mod 65536 of
out[b, t], which for the 1985 non-wrapping rows is a single strided
(diagonal) view assignment; the 63 wrapping rows are split copies.
The bias add (exact fp32) also folds into the host scatter:
out band row = pred_row + b.

Device per core: load [W.T | x.T] packed bf16 (6 k-tiles of 128 rows,
one DMA each so matmuls start after the first ~1.8us), 96 bf16 matmuls
(128x128x512, k-outer over 8 concurrent PSUM banks so accumulation
overlaps the remaining loads), DVE PSUM->SBUF copies (fp32->bf16), and
4 per-row-block band stores.  ~6 MB of HBM traffic and ~20.5us of PE
time per core, vs 134 MB of DMA in the write-the-zeros formulation.

The walrus build only leaves room for ONE sync-wait per compute
instruction; _split_multi_waits() hoists extra waits into same-queue
NOPs (same-queue waits execute in order, so this is semantics-neutral).
"""

import numpy as np

import bass_rust
import concourse.bass as bass
import concourse.mybir as mybir
import concourse.tile as tile
from concourse.bass_utils import run_bass_kernel_spmd

F32 = mybir.dt.float32
BF16 = mybir.dt.bfloat16
NP_BF16 = mybir.dt.np(mybir.dt.bfloat16)


def _split_multi_waits(nc):
    """Walrus in this toolchain only leaves ONE sync-wait slot per
    instruction.  Tile's tail drain waits on every semaphore lane it used,
    which fails codegen.  Hoist all-but-one wait of any multi-wait
    instruction into single-wait NOPs on the same engine queue immediately
    before it - semantically identical (same-queue waits execute in order).
    """
    eng_by_type = {
        mybir.EngineType.SP: nc.sync,
        mybir.EngineType.PE: nc.tensor,
        mybir.EngineType.Activation: nc.scalar,
        mybir.EngineType.Pool: nc.gpsimd,
        mybir.EngineType.DVE: nc.vector,
    }
    tail_bb = nc.cur_bb.bb
    for f in nc.m.functions:
        for bb in f.blocks:
            il = bb.instructions
            i = 0
            while i < len(il):
                ins = il[i]
                si = getattr(ins, "sync_info", None)
                if si is not None and len(si.on_wait) > 1:
                    waits = list(si.on_wait)
                    for w in waits[:-1]:
                        nop = eng_by_type[ins.engine].nop(nofuse=True).ins
                        tail_bb.instructions.remove(nop)
                        nop.sync_info = bass_rust.SyncInfo(
                            on_wait=[w], on_update=[])
                        il.insert(i, nop)
                        i += 1
                    ins.sync_info = bass_rust.SyncInfo(
                        on_wait=[waits[-1]], on_update=list(si.on_update))
                i += 1


# Problem shapes (hardcoded per contract).
B = 2
S = 2048
D_IN = 768
MAX_SPAN = 64
SPAN_DIM = 32
N_OUT = MAX_SPAN * SPAN_DIM  # 2048
N_CORES = 8
CHUNKS = 4                   # seq chunks per batch (B * CHUNKS == N_CORES)
ROWS = S // CHUNKS           # 512 rows per core


def build_nc(rows=ROWS, d_in=D_IN, n_out=N_OUT, repeats=1, nw=512):
    """Single-core Bass program (shared by all 8 cores via SPMD).

    Inputs (per core):
      wx : (d_in, n_out + rows) bf16, packed [W.T | x_chunk.T].
    Output:
      out: (rows, n_out) bf16 = pred = x_chunk @ W.T (no bias; host adds it).
    """
    kt = d_in // 128             # 6 contraction tiles
    mblk = rows // 128           # 4 row blocks
    nchunk = n_out // nw
    wcols = n_out + rows         # 2560
    half_mb = mblk // 2          # row blocks per PSUM generation

    nc = bass.Bass()
    wx = nc.dram_tensor("wx", [d_in, wcols], BF16, kind="ExternalInput")
    out = nc.dram_tensor("out", [rows, n_out], BF16, kind="ExternalOutput")

    wx_r = wx.rearrange("(k p) m -> p k m", p=128)    # (128, kt, wcols)
    out_r = out.rearrange("(mb p) c -> p mb c", p=128)  # (128, mblk, n_out)

    with tile.TileContext(nc) as tc:
        with (
            tc.tile_pool(name="wxp", bufs=2) as wxpool,
            tc.tile_pool(name="pred", bufs=2) as ppool,
            tc.tile_pool(name="psum", bufs=8, space="PSUM") as pspool,
        ):
            for _rep in range(repeats):
                # Per-k-tile loads so the first matmul sweep can start
                # after ~1/6 of the load, overlapping the rest.  Split
                # across two DMA rings (scalar HWDGE + gpsimd SWDGE) so
                # k-tiles land ~2x faster during the fill; stores own the
                # sync ring exclusively, so cross-repeat load prefetch is
                # never FIFO-blocked behind a store.
                wx_sb = wxpool.tile([128, kt, wcols], BF16)
                for k in range(kt):
                    eng = nc.scalar if k % 2 == 0 else nc.gpsimd
                    eng.dma_start(wx_sb[:, k, :], wx_r[:, k, :])

                pred = ppool.tile([128, mblk, n_out], BF16)
                # Two generations of 8 concurrent PSUM banks; k-outer so
                # accumulation for all 8 chunks proceeds as k-tiles land.
                for half in range(2):
                    pss = [pspool.tile([128, nw], F32, name="ps")
                           for _ in range(half_mb * nchunk)]
                    for k in range(kt):
                        for mi in range(half_mb):
                            mb = half * half_mb + mi
                            cs = n_out + mb * 128
                            for n in range(nchunk):
                                nc.tensor.matmul(
                                    pss[mi * nchunk + n][:],
                                    wx_sb[:, k, cs:cs + 128],
                                    wx_sb[:, k, n * nw:(n + 1) * nw],
                                    start=(k == 0),
                                    stop=(k == kt - 1),
                                )
                    for mi in range(half_mb):
                        mb = half * half_mb + mi
                        for n in range(nchunk):
                            nc.vector.tensor_copy(
                                pred[:, mb, n * nw:(n + 1) * nw],
                                pss[mi * nchunk + n][:])
                        # Band store for this 128-row block (4KB/partition).
                        nc.sync.dma_start(out_r[:, mb, :], pred[:, mb, :])

    _split_multi_waits(nc)
    return nc


_CACHE = {}


def _get_nc():
    if "nc" not in _CACHE:
        _CACHE["nc"] = build_nc()
    return _CACHE["nc"]


def make_in_maps(x, W, b):
    """Host-side sharding: per-core packed [W.T | x_chunk.T] bf16."""
    x = np.asarray(x)
    W = np.asarray(W)
    WT = np.ascontiguousarray(W.T).astype(NP_BF16)    # (768, 2048)
    in_maps = []
    for c in range(N_CORES):
        bi, cc = divmod(c, CHUNKS)
        xs = x[bi, cc * ROWS:(cc + 1) * ROWS, :]
        wxc = np.empty((D_IN, N_OUT + ROWS), NP_BF16)
        wxc[:, :N_OUT] = WT
        wxc[:, N_OUT:] = np.ascontiguousarray(xs.T).astype(NP_BF16)
        in_maps.append({"wx": wxc})
    return in_maps


def unshard(results, b):
    """Scatter each core's dense band into the zero-filled full output.

    Row t's band occupies flat columns [32*t, 32*t+2048) mod 65536 of
    out[bi, t]; rows 0..1984 never wrap, so they're one strided
    (diagonal-view) assignment per batch.  Bias is added here in fp32.
    """
    b = np.asarray(b, dtype=np.float32)
    row_f = S * SPAN_DIM                       # 65536 floats per row
    nowrap = (row_f - N_OUT) // SPAN_DIM + 1   # 1985 non-wrapping rows
    out = np.zeros((B, S, S, SPAN_DIM), np.float32)
    for bi in range(B):
        band = np.concatenate(
            [np.asarray(results[bi * CHUNKS + cc]["out"]).astype(np.float32)
             for cc in range(CHUNKS)], axis=0)          # (2048, 2048)
        band += b[None, :]
        flat = out[bi].reshape(S * row_f)
        dv = np.lib.stride_tricks.as_strided(
            flat, shape=(nowrap, N_OUT),
            strides=((row_f + SPAN_DIM) * 4, 4))
        dv[:] = band[:nowrap]
        for t in range(nowrap, S):
            c0 = SPAN_DIM * t
            n1 = row_f - c0
            row = flat[t * row_f:(t + 1) * row_f]
            row[c0:] = band[t, :n1]
            row[:N_OUT - n1] = band[t, n1:]
    return out


def _run(nc, in_maps):
    return run_bass_kernel_spmd(nc, in_maps, list(range(N_CORES))).results


def kernel(x, W, b):
    x = np.asarray(x)
    W = np.asarray(W)
    b = np.asarray(b)
    nc = _get_nc()
    res = _run(nc, make_in_maps(x, W, b))
    return unshard(res, b)
